# revision 6
# baseline (speedup 1.0000x reference)
"""NonLocalBlock (single-head attention, N=HW=4096, d=128) on 8 trn2 cores.

Sharding: data-parallel over batch (B=8) - one batch element per NeuronCore.

Design notes (vs. the 192us baseline):
  - x is loaded ONCE as fp16 (2MB instead of 6MB), split over 3 DMA queues.
  - theta/phi/g weights and activations are fp16: S and y matmuls both run
    at full PE rate (the old f32r S matmuls streamed at 2 cyc/col and were
    the EXP-chain pacer).
  - phi's bias is dropped (softmax over keys is invariant to it); theta's
    bias+scale are fused into its PSUM extraction.
  - theta carries A=2^7*log2(e) so 5 of 32 chunks per quarter (q>=1)
    compute exp on the DVE via the Schraudolph bit trick: i16 =
    clamp(S' + B, 0) reinterpreted as bf16 IS e^(S-40) to ~2-3%. Those
    chunks' S matmuls land in the "wy" PSUM tag so the Scalar engine's
    s-slot rotation never couples to DVE latency.
  - Column sums use TWO accumulators - 8 chunks/quarter accumulate on
    GpSimd, the rest on DVE - merged for free in the sums matmul.
  - theta/phi/g live in PER-BLOCK tiles: matmul weight (lhsT) reads get
    conservative (whole-tile) dependencies, so a shared tile would stall
    early S/y matmuls on unrelated later extracts (measured +8us).
  - Each quarter's epilogue PE work (sums matmul, wW projection) is
    DEFERRED into the next quarter's chunk loop: the in-order PE queue
    must never hold a matmul whose DVE-produced inputs (acc/ytn) are not
    ready, or the S-matmul stream behind it stalls the EXP chain.
  - Normalization is commuted BEFORE the wW projection (ytn = yu*recip in
    fp16); each quarter's output is extracted once and accumulated onto
    the pre-stored residual (x + bW', folded on host, one DRAM->DRAM DMA).
    The LAST quarter uses plain stores (residual added on DVE from a
    pre-loaded xpb slice): a trailing SWDGE accumulate costs ~7us in
    CCE + drain at the ramped-down tail.
  - y matmuls are emitted with a 2-chunk lag so a late g chunk can never
    head-of-line-block the S matmuls.
"""

import numpy as np
from contextlib import ExitStack

import concourse.bass as bass
import concourse.mybir as mybir
import concourse.tile as tile
from concourse import bacc

P = 128          # partitions / inter channels
C = 256          # input channels
F32 = mybir.dt.float32
F32R = mybir.dt.float32r
F16 = mybir.dt.float16
BF16 = mybir.dt.bfloat16
I16 = mybir.dt.int16
AF = mybir.ActivationFunctionType
ALU = mybir.AluOpType
CSHIFT = 40.0    # global score shift before exp (softmax-invariant)
A_SCALE = 184.66496414625282          # 2^7 * log2(e); theta pre-scale
B_SHIFT = 16256.0 - 5.5 - CSHIFT * A_SCALE  # Schraudolph offset (C=5.5)

B_FULL = 8
H_FULL = 64
W_FULL = 64
N_FULL = H_FULL * W_FULL

WARM_MMS = 40    # junk matmuls to trip the PE HAM to 8/8 before real work
DVE_EXP = {5, 11, 17, 23, 29}   # chunks whose exp runs on DVE (q>=1)
GP_ACC = {2, 6, 10, 14, 18, 22, 26, 30}  # chunks col-summed on GpSimd


def build_nc(N=N_FULL, NQ=1024):
    """Build the single-core Bass module (SPMD: same NEFF on all 8 cores)."""
    MC = N // P                   # 32 key chunks
    NQn = N // NQ                 # 4 query quarters
    NB = NQ // 512                # 2 512-col blocks per quarter
    NCB = N // 1024               # 4 1024-col proj blocks

    nc = bacc.Bacc("TRN2", target_bir_lowering=False, debug=False)

    xh_d = nc.dram_tensor("xh", [C, N], F16, kind="ExternalInput").ap()
    xpb_d = nc.dram_tensor("xpb", [C, N], F32R, kind="ExternalInput").ap()
    wtT_d = nc.dram_tensor("wtT", [P, 2 * P], F16, kind="ExternalInput").ap()
    wpT_d = nc.dram_tensor("wpT", [P, 2 * P], F16, kind="ExternalInput").ap()
    wgT_d = nc.dram_tensor("wgT", [P, 2 * P], F16, kind="ExternalInput").ap()
    wWT_d = nc.dram_tensor("wWT", [P, C], F16, kind="ExternalInput").ap()
    bt_d = nc.dram_tensor("bt", [P, 1], F32, kind="ExternalInput").ap()
    out_d = nc.dram_tensor("out", [C, N], F32R, kind="ExternalOutput").ap()

    xh_v = xh_d.rearrange("(k p) n -> k p n", p=P)
    xpb_v = xpb_d.rearrange("(k p) n -> k p n", p=P)
    out_v = out_d.rearrange("(k p) n -> k p n", p=P)

    with tile.TileContext(nc) as tc, ExitStack() as ctx:
        const = ctx.enter_context(tc.tile_pool(name="const", bufs=1))
        big = ctx.enter_context(tc.tile_pool(name="big", bufs=1))
        work = ctx.enter_context(tc.tile_pool(name="work", bufs=1))
        ps = ctx.enter_context(tc.tile_pool(name="ps", bufs=1, space="PSUM"))

        # ---- constants ----
        wtT_sb = const.tile([P, 2, P], F16, name="wtT_sb")
        wpT_sb = const.tile([P, 2, P], F16, name="wpT_sb")
        wgT_sb = const.tile([P, 2, P], F16, name="wgT_sb")
        wWT_sb = const.tile([P, C], F16, name="wWT_sb")
        bt_sb = const.tile([P, 1], F32, name="bt_sb")
        ones_sb = const.tile([P, P], BF16, name="ones_sb")
        cshift_sb = const.tile([P, 1], F32, name="cshift_sb")
        nc.vector.memset(cshift_sb[:], -CSHIFT)
        nc.vector.memset(ones_sb[:], 1.0)

        # weights on the scalar queue (small; lands before first use)
        nc.scalar.dma_start(wtT_sb[:], wtT_d.rearrange("p (k i) -> p k i", k=2))
        nc.scalar.dma_start(wpT_sb[:], wpT_d.rearrange("p (k i) -> p k i", k=2))
        nc.scalar.dma_start(bt_sb[:], bt_d)
        nc.scalar.dma_start(wgT_sb[:], wgT_d.rearrange("p (k i) -> p k i", k=2))
        nc.scalar.dma_start(wWT_sb[:], wWT_d)

        # x halves on two parallel queues: k=0 via sync, k=1 via gpsimd
        xh_sb = big.tile([P, 2, N], F16, name="xh_sb")
        for b in range(8):
            sl = slice(b * 512, (b + 1) * 512)
            nc.sync.dma_start(xh_sb[:, 0, sl], xh_v[0, :, sl])
        for b in range(8):
            sl = slice(b * 512, (b + 1) * 512)
            nc.gpsimd.dma_start(xh_sb[:, 1, sl], xh_v[1, :, sl])

        # residual (+bW', folded on host) pre-store: one DRAM->DRAM copy,
        # queued behind the x stream on the sync queue. The LAST quarter is
        # stored plainly instead, with its residual slice added from SBUF.
        NPRE = N - NQ
        nc.sync.dma_start(out_d[:, 0:NPRE], xpb_d[:, 0:NPRE])
        xpbq_sb = big.tile([P, 2, NQ], F32R, name="xpbq_sb")
        for k in range(2):
            nc.sync.dma_start(xpbq_sb[:, k], xpb_v[k, :, NPRE:N])

        # per-block tiles (single writer each -> exact matmul weight deps)
        th_t = [big.tile([P, 1024], F16, name=f"th{i}") for i in range(NCB)]
        ph_t = [big.tile([P, 1024], F16, name=f"ph{i}") for i in range(NCB)]
        g_t = [big.tile([P, 4, P], BF16, name=f"g{i}") for i in range(8)]

        # ---- PE warmup: trip HAM to 8/8 while the first x blocks land ----
        warm_ps = ps.tile([P, P], F32, tag="s", bufs=2, name="warm_ps")
        for _ in range(WARM_MMS):
            nc.tensor.matmul(warm_ps[:], ones_sb[:], ones_sb[:],
                             start=True, stop=True, skip_group_check=True)

        def proj_block(dst_t, w_sb, cb, tag, engine):
            """theta/phi projection for 1024-col block cb; extract on
            `engine` ('act'|'dve'); theta also gets (+bt)*A fused."""
            is_th = dst_t is th_t
            p_ps = ps.tile([P, 1024], F32, tag=tag, bufs=2 if tag == "s"
                           else None, name="p_ps")
            for h in range(2):
                hsl = slice(cb * 1024 + h * 512, cb * 1024 + (h + 1) * 512)
                for k in range(2):
                    nc.tensor.matmul(p_ps[:, h * 512:(h + 1) * 512],
                                     w_sb[:, k], xh_sb[:, k, hsl],
                                     start=(k == 0), stop=(k == 1))
            if is_th:
                nc.vector.tensor_scalar(dst_t[cb][:], p_ps[:],
                                        bt_sb[:, 0:1], A_SCALE,
                                        op0=ALU.add, op1=ALU.mult)
            elif engine == "act":
                nc.scalar.activation(dst_t[cb][:], p_ps[:], AF.Identity)
            else:
                nc.vector.tensor_copy(dst_t[cb][:], p_ps[:])

        def g_block(t):
            """g for key chunks 4t..4t+3 (x block t)."""
            g_ps = ps.tile([P, 4, P], F32, tag="wy", name="g_ps")
            for j in range(4):
                msl = slice((4 * t + j) * P, (4 * t + j + 1) * P)
                for k in range(2):
                    nc.tensor.matmul(g_ps[:, j], xh_sb[:, k, msl],
                                     wgT_sb[:, k], start=(k == 0),
                                     stop=(k == 1))
            nc.vector.tensor_copy(g_t[t][:], g_ps[:])

        # critical first blocks: th0 (extract on DVE) + ph0 (extract on ACT)
        # run in the "s" slots before the attention chain starts; g0 too so
        # the first y matmuls never stall the in-order PE queue.
        proj_block(th_t, wtT_sb, 0, "s", "dve")
        proj_block(ph_t, wpT_sb, 0, "s", "act")
        g_block(0)

        # deferred prep, interleaved into q0's chunk loop (emission order
        # == PE queue order; x-arrival times comfortably precede these)
        prep = {1: lambda: g_block(1),
                3: lambda: proj_block(ph_t, wpT_sb, 1, "wy", "dve"),
                5: lambda: g_block(2),
                7: lambda: g_block(3),
                9: lambda: proj_block(ph_t, wpT_sb, 2, "wy", "dve"),
                11: lambda: g_block(4),
                13: lambda: g_block(5),
                15: lambda: proj_block(ph_t, wpT_sb, 3, "wy", "dve"),
                17: lambda: g_block(6),
                19: lambda: g_block(7),
                21: lambda: proj_block(th_t, wtT_sb, 1, "wy", "dve"),
                23: lambda: proj_block(th_t, wtT_sb, 2, "wy", "dve"),
                25: lambda: proj_block(th_t, wtT_sb, 3, "wy", "dve")}

        YLAG = 2  # y-matmul emission lag (chunks), protects PE queue order

        def epi_sums(E):
            """Deferred epilogue of quarter E: sums matmul (merging both
            accumulators), reciprocal, and the fp16 normalized yt."""
            sum_ps = ps.tile([P, NQ], F32, tag="wy", name="sum_ps")
            for b in range(NB):
                bsl = slice(b * 512, (b + 1) * 512)
                nc.tensor.matmul(sum_ps[:, bsl], ones_sb[:],
                                 E["acc_d"][:, bsl], start=True, stop=False,
                                 skip_group_check=True)
                nc.tensor.matmul(sum_ps[:, bsl], ones_sb[:],
                                 E["acc_g"][:, bsl], start=False, stop=True,
                                 skip_group_check=True)
            recip_sb = work.tile([P, NQ], F32, tag="recip", bufs=2,
                                 name="recip_sb")
            nc.vector.reciprocal_approx_fast(recip_sb[:], sum_ps[:])
            nc.vector.tensor_mul(E["ytn"][:], E["yt"][:], recip_sb[:])

        def epi_wy(E, h):
            """Deferred epilogue: wW projection half h + accumulate-store."""
            q = E["q"]
            wy_ps = ps.tile([P, NQ], F32, tag="wy", name="wy_ps")
            for b in range(NB):
                bsl = slice(b * 512, (b + 1) * 512)
                nc.tensor.matmul(wy_ps[:, bsl], wWT_sb[:, h * P:(h + 1) * P],
                                 E["ytn"][:, bsl], start=True, stop=True)
            o_sb = work.tile([P, NQ], F32R, tag="ow", bufs=2, name="o_sb")
            nc.vector.tensor_copy(o_sb[:], wy_ps[:])
            nc.gpsimd.dma_start(out_v[h, :, q * NQ:(q + 1) * NQ], o_sb[:],
                                accum_op=ALU.add)

        pending = None  # previous quarter's epilogue state

        for q in range(NQn):
            last = (q == NQn - 1)
            y_ps = ps.tile([P, NQ], F32, tag="y", name="y_ps")
            acc_d = work.tile([P, NQ], BF16, tag="acc_d", bufs=2, name="acc_d")
            acc_g = work.tile([P, NQ], BF16, tag="acc_g", bufs=2, name="acc_g")
            exps = {}
            gp_first = min(GP_ACC)

            def y_mms(mc):
                e = exps.pop(mc)
                for b in range(NB):
                    bsl = slice(b * 512, (b + 1) * 512)
                    nc.tensor.matmul(y_ps[:, bsl], g_t[mc // 4][:, mc % 4],
                                     e[:, bsl],
                                     start=(mc == 0), stop=(mc == MC - 1),
                                     skip_group_check=True)

            for mc in range(MC):
                on_dve = q > 0 and mc in DVE_EXP
                s_ps = ps.tile([P, NQ], F32, tag="wy" if on_dve else "s",
                               bufs=None if on_dve else 2, name="s_ps")
                msl = slice((mc % 8) * P, (mc % 8 + 1) * P)
                for b in range(NB):
                    bsl = slice(b * 512, (b + 1) * 512)
                    nc.tensor.matmul(
                        s_ps[:, bsl], ph_t[mc // 8][:, msl],
                        th_t[q][:, b * 512:(b + 1) * 512],
                        start=True, stop=True)
                exp_sb = work.tile([P, NQ], BF16, tag="exp", bufs=12,
                                   name="exp_sb")
                if on_dve:
                    # Schraudolph: bf16 bits of e^(S-40) via one DVE op
                    nc.vector.tensor_scalar(exp_sb[:].bitcast(I16), s_ps[:],
                                            B_SHIFT, 0.0,
                                            op0=ALU.add, op1=ALU.max)
                else:
                    nc.scalar.activation(exp_sb[:], s_ps[:], AF.Exp,
                                         bias=cshift_sb[:, 0:1],
                                         scale=1.0 / A_SCALE)
                exps[mc] = exp_sb
                if mc in GP_ACC:
                    if mc == gp_first:
                        nc.gpsimd.tensor_copy(acc_g[:], exp_sb[:])
                    else:
                        nc.gpsimd.tensor_add(acc_g[:], acc_g[:], exp_sb[:])
                elif mc == 0:
                    nc.vector.tensor_copy(acc_d[:], exp_sb[:])
                else:
                    nc.vector.tensor_add(acc_d[:], acc_d[:], exp_sb[:])
                if mc >= YLAG:
                    y_mms(mc - YLAG)
                if q == 0 and mc in prep:
                    prep[mc]()
                if pending is not None:
                    if mc == 1:
                        epi_sums(pending)
                    elif mc == 3:
                        epi_wy(pending, 0)
                    elif mc == 5:
                        epi_wy(pending, 1)
                        pending = None
            for mc in range(MC - YLAG, MC):
                y_mms(mc)

            # y extraction frees the y banks for the next quarter; the rest
            # of the epilogue is deferred into the next quarter's loop.
            yt_sb = work.tile([P, NQ], F32R, tag="yt", bufs=2, name="yt_sb")
            ytn_sb = work.tile([P, NQ], F16, tag="ytn", bufs=2, name="ytn_sb")
            if last:
                nc.scalar.activation(yt_sb[:], y_ps[:], AF.Identity)
            else:
                nc.vector.tensor_copy(yt_sb[:], y_ps[:])
            pending = {"q": q, "acc_d": acc_d, "acc_g": acc_g,
                       "yt": yt_sb, "ytn": ytn_sb}

        # ---- tail: the last quarter's epilogue, plain stores ----
        E = pending
        q = E["q"]
        epi_sums(E)
        for h in range(2):
            wy_ps = ps.tile([P, NQ], F32, tag="wy", name="wy_ps")
            for b in range(NB):
                bsl = slice(b * 512, (b + 1) * 512)
                nc.tensor.matmul(wy_ps[:, bsl], wWT_sb[:, h * P:(h + 1) * P],
                                 E["ytn"][:, bsl], start=True, stop=True)
            for b in range(NB):
                bsl = slice(b * 512, (b + 1) * 512)
                o_sb = work.tile([P, 512], F32R, tag="o", bufs=4, name="o_sb")
                nc.vector.tensor_add(o_sb[:], wy_ps[:, bsl],
                                     xpbq_sb[:, h, bsl])
                nc.sync.dma_start(
                    out_v[h, :, q * NQ + b * 512: q * NQ + (b + 1) * 512],
                    o_sb[:])

    nc.compile()
    return nc


_CACHE = {}


def _built(key=(N_FULL, 1024)):
    if key not in _CACHE:
        _CACHE[key] = build_nc(*key)
    return _CACHE[key]


def make_in_maps(x, wg, bg, wt, bt, wp, bp, wW, bW):
    """Host-side prep: per-core input dicts (core b <- batch b)."""
    x = np.asarray(x, np.float32)
    B, C_, H, W = x.shape
    N = H * W
    xf = np.ascontiguousarray(x.reshape(B, C_, N))
    wg, bg, wt, bt, wp, bp, wW, bW = [
        np.asarray(a, np.float32) for a in (wg, bg, wt, bt, wp, bp, wW, bW)]

    def pack(w):  # (128, C) conv weight -> partition-major lhsT, fp16
        return np.ascontiguousarray(
            w.T.reshape(2, P, P).transpose(1, 0, 2).reshape(P, 2 * P)
        ).astype(np.float16)

    bWp = (wW @ bg + bW).astype(np.float32)       # fold bg into bW
    shared = {
        "wtT": pack(wt), "wpT": pack(wp), "wgT": pack(wg),
        "wWT": np.ascontiguousarray(wW.T).astype(np.float16),
        "bt": bt.reshape(P, 1).copy(),
    }
    return [{"xh": np.ascontiguousarray(xf[b]).astype(np.float16),
             "xpb": np.ascontiguousarray(xf[b] + bWp[:, None]),
             **shared} for b in range(B)]


def kernel(x, wg, bg, wt, bt, wp, bp, wW, bW):
    from concourse.bass_utils import run_bass_kernel_spmd

    B, C_, H, W = np.asarray(x).shape
    in_maps = make_in_maps(x, wg, bg, wt, bt, wp, bp, wW, bW)
    nc = _built()
    res = run_bass_kernel_spmd(nc, in_maps, core_ids=list(range(B)))
    out = np.stack([res.results[b]["out"] for b in range(B)])
    return out.reshape(B, C_, H, W).astype(np.float32)


# revision 9
# speedup vs baseline: 1.2271x; 1.2271x over previous
"""NonLocalBlock (single-head attention, N=HW=4096, d=128) on 8 trn2 cores.

Sharding: data-parallel over batch (B=8) - one batch element per NeuronCore.

Design notes (vs. the 192us baseline):
  - x is loaded ONCE as fp16 (2MB instead of 6MB), split over 3 DMA queues.
  - theta/phi/g weights and activations are fp16: S and y matmuls both run
    at full PE rate (the old f32r S matmuls streamed at 2 cyc/col and were
    the EXP-chain pacer).
  - phi's bias is dropped (softmax over keys is invariant to it); theta's
    bias+scale are fused into its PSUM extraction.
  - theta carries A=2^7*log2(e) so 5 of 32 chunks per quarter (q>=1)
    compute exp on the DVE via the Schraudolph bit trick: i16 =
    clamp(S' + B, 0) reinterpreted as bf16 IS e^(S-40) to ~2-3%. Those
    chunks' S matmuls land in the "wy" PSUM tag so the Scalar engine's
    s-slot rotation never couples to DVE latency.
  - Column sums use TWO accumulators - 8 chunks/quarter accumulate on
    GpSimd, the rest on DVE - merged for free in the sums matmul.
  - theta/phi/g live in PER-BLOCK tiles: matmul weight (lhsT) reads get
    conservative (whole-tile) dependencies, so a shared tile would stall
    early S/y matmuls on unrelated later extracts (measured +8us).
  - Each quarter's epilogue PE work (sums matmul, wW projection) is
    DEFERRED into the next quarter's chunk loop: the in-order PE queue
    must never hold a matmul whose DVE-produced inputs (acc/ytn) are not
    ready, or the S-matmul stream behind it stalls the EXP chain.
  - Normalization is commuted BEFORE the wW projection (ytn = yu*recip in
    fp16); each quarter's output is extracted once and accumulated onto
    the pre-stored residual (x + bW', folded on host, one DRAM->DRAM DMA).
    The LAST quarter uses plain stores (residual added on DVE from a
    pre-loaded xpb slice): a trailing SWDGE accumulate costs ~7us in
    CCE + drain at the ramped-down tail.
  - y matmuls are emitted with a 2-chunk lag so a late g chunk can never
    head-of-line-block the S matmuls.
"""

import numpy as np
from contextlib import ExitStack

import concourse.bass as bass
import concourse.mybir as mybir
import concourse.tile as tile
from concourse import bacc

P = 128          # partitions / inter channels
C = 256          # input channels
F32 = mybir.dt.float32
F32R = mybir.dt.float32r
F16 = mybir.dt.float16
BF16 = mybir.dt.bfloat16
I16 = mybir.dt.int16
AF = mybir.ActivationFunctionType
ALU = mybir.AluOpType
CSHIFT = 40.0    # global score shift before exp (softmax-invariant)
A_SCALE = 184.66496414625282          # 2^7 * log2(e); theta pre-scale
B_SHIFT = 16256.0 - 5.5 - CSHIFT * A_SCALE  # Schraudolph offset (C=5.5)

B_FULL = 8
H_FULL = 64
W_FULL = 64
N_FULL = H_FULL * W_FULL

WARM_MMS = 40    # junk matmuls to trip the PE HAM to 8/8 before real work
DVE_EXP = {8, 16, 24}   # chunks whose exp runs on DVE (q>=1)
# GpSimd col-sum offload measured counterproductive: its tensor_tensor adds
# run at ~2.4us AND slow concurrent DVE adds 700->1110ns (SBUF port
# contention), so all column sums stay on DVE.
GP_ACC = frozenset()


def build_nc(N=N_FULL, NQ=1024):
    """Build the single-core Bass module (SPMD: same NEFF on all 8 cores)."""
    MC = N // P                   # 32 key chunks
    NQn = N // NQ                 # 4 query quarters
    NB = NQ // 512                # 2 512-col blocks per quarter
    NCB = N // 1024               # 4 1024-col proj blocks

    nc = bacc.Bacc("TRN2", target_bir_lowering=False, debug=False)

    xh_d = nc.dram_tensor("xh", [C, N], F16, kind="ExternalInput").ap()
    xpb_d = nc.dram_tensor("xpb", [C, N], F32R, kind="ExternalInput").ap()
    wtT_d = nc.dram_tensor("wtT", [P, 2 * P], F16, kind="ExternalInput").ap()
    wpT_d = nc.dram_tensor("wpT", [P, 2 * P], F16, kind="ExternalInput").ap()
    wgT_d = nc.dram_tensor("wgT", [P, 2 * P], F16, kind="ExternalInput").ap()
    wWT_d = nc.dram_tensor("wWT", [P, C], F16, kind="ExternalInput").ap()
    bt_d = nc.dram_tensor("bt", [P, 1], F32, kind="ExternalInput").ap()
    out_d = nc.dram_tensor("out", [C, N], F32R, kind="ExternalOutput").ap()

    xh_v = xh_d.rearrange("(k p) n -> k p n", p=P)
    xpb_v = xpb_d.rearrange("(k p) n -> k p n", p=P)
    out_v = out_d.rearrange("(k p) n -> k p n", p=P)

    with tile.TileContext(nc) as tc, ExitStack() as ctx:
        const = ctx.enter_context(tc.tile_pool(name="const", bufs=1))
        big = ctx.enter_context(tc.tile_pool(name="big", bufs=1))
        work = ctx.enter_context(tc.tile_pool(name="work", bufs=1))
        ps = ctx.enter_context(tc.tile_pool(name="ps", bufs=1, space="PSUM"))

        # ---- constants ----
        wtT_sb = const.tile([P, 2, P], F16, name="wtT_sb")
        wpT_sb = const.tile([P, 2, P], F16, name="wpT_sb")
        wgT_sb = const.tile([P, 2, P], F16, name="wgT_sb")
        wWT_sb = const.tile([P, C], F16, name="wWT_sb")
        bt_sb = const.tile([P, 1], F32, name="bt_sb")
        ones_sb = const.tile([P, P], BF16, name="ones_sb")
        cshift_sb = const.tile([P, 1], F32, name="cshift_sb")
        nc.vector.memset(cshift_sb[:], -CSHIFT)
        nc.vector.memset(ones_sb[:], 1.0)

        # weights on the scalar queue (small; lands before first use)
        nc.scalar.dma_start(wtT_sb[:], wtT_d.rearrange("p (k i) -> p k i", k=2))
        nc.scalar.dma_start(wpT_sb[:], wpT_d.rearrange("p (k i) -> p k i", k=2))
        nc.scalar.dma_start(bt_sb[:], bt_d)
        nc.scalar.dma_start(wgT_sb[:], wgT_d.rearrange("p (k i) -> p k i", k=2))
        nc.scalar.dma_start(wWT_sb[:], wWT_d)

        # x halves on two parallel queues: k=0 via sync, k=1 via gpsimd
        xh_sb = big.tile([P, 2, N], F16, name="xh_sb")
        for b in range(8):
            sl = slice(b * 512, (b + 1) * 512)
            nc.sync.dma_start(xh_sb[:, 0, sl], xh_v[0, :, sl])
        for b in range(8):
            sl = slice(b * 512, (b + 1) * 512)
            nc.gpsimd.dma_start(xh_sb[:, 1, sl], xh_v[1, :, sl])

        # residual (+bW', folded on host) pre-store: one DRAM->DRAM copy,
        # queued behind the x stream on the sync queue. The LAST quarter is
        # stored plainly instead, with its residual slice added from SBUF.
        NPRE = N - NQ
        nc.sync.dma_start(out_d[:, 0:NPRE], xpb_d[:, 0:NPRE])
        xpbq_sb = big.tile([P, 2, NQ], F32R, name="xpbq_sb")
        for k in range(2):
            nc.sync.dma_start(xpbq_sb[:, k], xpb_v[k, :, NPRE:N])

        # per-block tiles (single writer each -> exact matmul weight deps)
        th_t = [big.tile([P, 1024], F16, name=f"th{i}") for i in range(NCB)]
        ph_t = [big.tile([P, 1024], F16, name=f"ph{i}") for i in range(NCB)]
        g_t = [big.tile([P, 4, P], BF16, name=f"g{i}") for i in range(8)]

        # ---- PE warmup: trip HAM to 8/8 while the first x blocks land ----
        warm_ps = ps.tile([P, P], F32, tag="s", bufs=2, name="warm_ps")
        for _ in range(WARM_MMS):
            nc.tensor.matmul(warm_ps[:], ones_sb[:], ones_sb[:],
                             start=True, stop=True, skip_group_check=True)

        def proj_block(dst_t, w_sb, cb, tag, engine):
            """theta/phi projection for 1024-col block cb; extract on
            `engine` ('act'|'dve'); theta also gets (+bt)*A fused."""
            is_th = dst_t is th_t
            p_ps = ps.tile([P, 1024], F32, tag=tag, bufs=2 if tag == "s"
                           else None, name="p_ps")
            for h in range(2):
                hsl = slice(cb * 1024 + h * 512, cb * 1024 + (h + 1) * 512)
                for k in range(2):
                    nc.tensor.matmul(p_ps[:, h * 512:(h + 1) * 512],
                                     w_sb[:, k], xh_sb[:, k, hsl],
                                     start=(k == 0), stop=(k == 1))
            if is_th:
                nc.vector.tensor_scalar(dst_t[cb][:], p_ps[:],
                                        bt_sb[:, 0:1], A_SCALE,
                                        op0=ALU.add, op1=ALU.mult)
            elif engine == "act":
                nc.scalar.activation(dst_t[cb][:], p_ps[:], AF.Identity)
            else:
                nc.vector.tensor_copy(dst_t[cb][:], p_ps[:])

        def g_block(t):
            """g for key chunks 4t..4t+3 (x block t)."""
            g_ps = ps.tile([P, 4, P], F32, tag="wy", name="g_ps")
            for j in range(4):
                msl = slice((4 * t + j) * P, (4 * t + j + 1) * P)
                for k in range(2):
                    nc.tensor.matmul(g_ps[:, j], xh_sb[:, k, msl],
                                     wgT_sb[:, k], start=(k == 0),
                                     stop=(k == 1))
            nc.vector.tensor_copy(g_t[t][:], g_ps[:])

        # critical first blocks: th0 (extract on DVE) + ph0 (extract on ACT)
        # run in the "s" slots before the attention chain starts; g0 too so
        # the first y matmuls never stall the in-order PE queue.
        proj_block(th_t, wtT_sb, 0, "s", "dve")
        proj_block(ph_t, wpT_sb, 0, "s", "act")
        g_block(0)

        # deferred prep, interleaved into q0's chunk loop (emission order
        # == PE queue order; x-arrival times comfortably precede these)
        prep = {1: lambda: g_block(1),
                3: lambda: proj_block(ph_t, wpT_sb, 1, "wy", "dve"),
                5: lambda: g_block(2),
                7: lambda: g_block(3),
                9: lambda: proj_block(ph_t, wpT_sb, 2, "wy", "dve"),
                11: lambda: g_block(4),
                13: lambda: g_block(5),
                15: lambda: proj_block(ph_t, wpT_sb, 3, "wy", "dve"),
                17: lambda: g_block(6),
                19: lambda: g_block(7),
                21: lambda: proj_block(th_t, wtT_sb, 1, "wy", "dve"),
                23: lambda: proj_block(th_t, wtT_sb, 2, "wy", "dve"),
                25: lambda: proj_block(th_t, wtT_sb, 3, "wy", "dve")}

        YLAG = 2  # y-matmul emission lag (chunks), protects PE queue order

        def epi_sums(E):
            """Deferred epilogue of quarter E: sums matmul (merging both
            accumulators), reciprocal, and the fp16 normalized yt."""
            sum_ps = ps.tile([P, NQ], F32, tag="wy", name="sum_ps")
            for b in range(NB):
                bsl = slice(b * 512, (b + 1) * 512)
                accs = [E["acc_d"]] + ([E["acc_g"]] if GP_ACC else [])
                for i, a in enumerate(accs):
                    nc.tensor.matmul(sum_ps[:, bsl], ones_sb[:], a[:, bsl],
                                     start=(i == 0), stop=(i == len(accs) - 1),
                                     skip_group_check=True)
            recip_sb = work.tile([P, NQ], F32, tag="recip", bufs=2,
                                 name="recip_sb")
            nc.vector.reciprocal_approx_fast(recip_sb[:], sum_ps[:])
            nc.vector.tensor_mul(E["ytn"][:], E["yt"][:], recip_sb[:])

        def epi_wy(E, h):
            """Deferred epilogue: wW projection half h + accumulate-store."""
            q = E["q"]
            wy_ps = ps.tile([P, NQ], F32, tag="wy", name="wy_ps")
            for b in range(NB):
                bsl = slice(b * 512, (b + 1) * 512)
                nc.tensor.matmul(wy_ps[:, bsl], wWT_sb[:, h * P:(h + 1) * P],
                                 E["ytn"][:, bsl], start=True, stop=True)
            o_sb = work.tile([P, NQ], F32R, tag="ow", bufs=2, name="o_sb")
            nc.vector.tensor_copy(o_sb[:], wy_ps[:])
            nc.gpsimd.dma_start(out_v[h, :, q * NQ:(q + 1) * NQ], o_sb[:],
                                accum_op=ALU.add)

        pending = None  # previous quarter's epilogue state

        for q in range(NQn):
            last = (q == NQn - 1)
            y_ps = ps.tile([P, NQ], F32, tag="y", name="y_ps")
            acc_d = work.tile([P, NQ], BF16, tag="acc_d", bufs=2, name="acc_d")
            acc_g = (work.tile([P, NQ], BF16, tag="acc_g", bufs=2,
                               name="acc_g") if GP_ACC else None)
            exps = {}
            gp_first = min(GP_ACC) if GP_ACC else None

            def y_mms(mc):
                e = exps.pop(mc)
                for b in range(NB):
                    bsl = slice(b * 512, (b + 1) * 512)
                    nc.tensor.matmul(y_ps[:, bsl], g_t[mc // 4][:, mc % 4],
                                     e[:, bsl],
                                     start=(mc == 0), stop=(mc == MC - 1),
                                     skip_group_check=True)

            for mc in range(MC):
                on_dve = q > 0 and mc in DVE_EXP
                s_ps = ps.tile([P, NQ], F32, tag="wy" if on_dve else "s",
                               bufs=None if on_dve else 2, name="s_ps")
                msl = slice((mc % 8) * P, (mc % 8 + 1) * P)
                for b in range(NB):
                    bsl = slice(b * 512, (b + 1) * 512)
                    nc.tensor.matmul(
                        s_ps[:, bsl], ph_t[mc // 8][:, msl],
                        th_t[q][:, b * 512:(b + 1) * 512],
                        start=True, stop=True)
                exp_sb = work.tile([P, NQ], BF16, tag="exp", bufs=12,
                                   name="exp_sb")
                if on_dve:
                    # Schraudolph: bf16 bits of e^(S-40) via one DVE op
                    nc.vector.tensor_scalar(exp_sb[:].bitcast(I16), s_ps[:],
                                            B_SHIFT, 0.0,
                                            op0=ALU.add, op1=ALU.max)
                else:
                    nc.scalar.activation(exp_sb[:], s_ps[:], AF.Exp,
                                         bias=cshift_sb[:, 0:1],
                                         scale=1.0 / A_SCALE)
                exps[mc] = exp_sb
                if mc in GP_ACC:
                    if mc == gp_first:
                        nc.gpsimd.tensor_copy(acc_g[:], exp_sb[:])
                    else:
                        nc.gpsimd.tensor_add(acc_g[:], acc_g[:], exp_sb[:])
                elif mc == 0:
                    nc.vector.tensor_copy(acc_d[:], exp_sb[:])
                else:
                    nc.vector.tensor_add(acc_d[:], acc_d[:], exp_sb[:])
                if mc >= YLAG:
                    y_mms(mc - YLAG)
                if q == 0 and mc in prep:
                    prep[mc]()
                if pending is not None:
                    if mc == 1:
                        epi_sums(pending)
                    elif mc == 3:
                        epi_wy(pending, 0)
                    elif mc == 5:
                        epi_wy(pending, 1)
                        pending = None
            for mc in range(MC - YLAG, MC):
                y_mms(mc)

            # y extraction frees the y banks for the next quarter; the rest
            # of the epilogue is deferred into the next quarter's loop.
            yt_sb = work.tile([P, NQ], F32R, tag="yt", bufs=2, name="yt_sb")
            ytn_sb = work.tile([P, NQ], F16, tag="ytn", bufs=2, name="ytn_sb")
            if last:
                nc.scalar.activation(yt_sb[:], y_ps[:], AF.Identity)
            else:
                nc.vector.tensor_copy(yt_sb[:], y_ps[:])
            pending = {"q": q, "acc_d": acc_d, "acc_g": acc_g,
                       "yt": yt_sb, "ytn": ytn_sb}

        # ---- tail: the last quarter's epilogue, plain stores ----
        E = pending
        q = E["q"]
        epi_sums(E)
        for h in range(2):
            wy_ps = ps.tile([P, NQ], F32, tag="wy", name="wy_ps")
            for b in range(NB):
                bsl = slice(b * 512, (b + 1) * 512)
                nc.tensor.matmul(wy_ps[:, bsl], wWT_sb[:, h * P:(h + 1) * P],
                                 E["ytn"][:, bsl], start=True, stop=True)
            for b in range(NB):
                bsl = slice(b * 512, (b + 1) * 512)
                o_sb = work.tile([P, 512], F32R, tag="o", bufs=4, name="o_sb")
                nc.vector.tensor_add(o_sb[:], wy_ps[:, bsl],
                                     xpbq_sb[:, h, bsl])
                nc.sync.dma_start(
                    out_v[h, :, q * NQ + b * 512: q * NQ + (b + 1) * 512],
                    o_sb[:])

    nc.compile()
    return nc


_CACHE = {}


def _built(key=(N_FULL, 1024)):
    if key not in _CACHE:
        _CACHE[key] = build_nc(*key)
    return _CACHE[key]


def make_in_maps(x, wg, bg, wt, bt, wp, bp, wW, bW):
    """Host-side prep: per-core input dicts (core b <- batch b)."""
    x = np.asarray(x, np.float32)
    B, C_, H, W = x.shape
    N = H * W
    xf = np.ascontiguousarray(x.reshape(B, C_, N))
    wg, bg, wt, bt, wp, bp, wW, bW = [
        np.asarray(a, np.float32) for a in (wg, bg, wt, bt, wp, bp, wW, bW)]

    def pack(w):  # (128, C) conv weight -> partition-major lhsT, fp16
        return np.ascontiguousarray(
            w.T.reshape(2, P, P).transpose(1, 0, 2).reshape(P, 2 * P)
        ).astype(np.float16)

    bWp = (wW @ bg + bW).astype(np.float32)       # fold bg into bW
    shared = {
        "wtT": pack(wt), "wpT": pack(wp), "wgT": pack(wg),
        "wWT": np.ascontiguousarray(wW.T).astype(np.float16),
        "bt": bt.reshape(P, 1).copy(),
    }
    return [{"xh": np.ascontiguousarray(xf[b]).astype(np.float16),
             "xpb": np.ascontiguousarray(xf[b] + bWp[:, None]),
             **shared} for b in range(B)]


def kernel(x, wg, bg, wt, bt, wp, bp, wW, bW):
    from concourse.bass_utils import run_bass_kernel_spmd

    B, C_, H, W = np.asarray(x).shape
    in_maps = make_in_maps(x, wg, bg, wt, bt, wp, bp, wW, bW)
    nc = _built()
    res = run_bass_kernel_spmd(nc, in_maps, core_ids=list(range(B)))
    out = np.stack([res.results[b]["out"] for b in range(B)])
    return out.reshape(B, C_, H, W).astype(np.float32)


# revision 15
# speedup vs baseline: 1.2476x; 1.0167x over previous
"""NonLocalBlock (single-head attention, N=HW=4096, d=128) on 8 trn2 cores.

Sharding: data-parallel over batch (B=8) - one batch element per NeuronCore.

Design notes (vs. the 192us baseline):
  - x is loaded ONCE as fp16 (2MB instead of 6MB), split over 3 DMA queues.
  - theta/phi/g weights and activations are fp16: S and y matmuls both run
    at full PE rate (the old f32r S matmuls streamed at 2 cyc/col and were
    the EXP-chain pacer).
  - phi's bias is dropped (softmax over keys is invariant to it); theta's
    bias+scale are fused into its PSUM extraction.
  - theta carries A=2^7*log2(e) so 5 of 32 chunks per quarter (q>=1)
    compute exp on the DVE via the Schraudolph bit trick: i16 =
    clamp(S' + B, 0) reinterpreted as bf16 IS e^(S-40) to ~2-3%. Those
    chunks' S matmuls land in the "wy" PSUM tag so the Scalar engine's
    s-slot rotation never couples to DVE latency.
  - Column sums use TWO accumulators - 8 chunks/quarter accumulate on
    GpSimd, the rest on DVE - merged for free in the sums matmul.
  - theta/phi/g live in PER-BLOCK tiles: matmul weight (lhsT) reads get
    conservative (whole-tile) dependencies, so a shared tile would stall
    early S/y matmuls on unrelated later extracts (measured +8us).
  - Each quarter's epilogue PE work (sums matmul, wW projection) is
    DEFERRED into the next quarter's chunk loop: the in-order PE queue
    must never hold a matmul whose DVE-produced inputs (acc/ytn) are not
    ready, or the S-matmul stream behind it stalls the EXP chain.
  - Normalization is commuted BEFORE the wW projection (ytn = yu*recip in
    fp16); each quarter's output is extracted once and accumulated onto
    the pre-stored residual (x + bW', folded on host, one DRAM->DRAM DMA).
    The LAST quarter uses plain stores (residual added on DVE from a
    pre-loaded xpb slice): a trailing SWDGE accumulate costs ~7us in
    CCE + drain at the ramped-down tail.
  - y matmuls are emitted with a 2-chunk lag so a late g chunk can never
    head-of-line-block the S matmuls.
"""

import numpy as np
from contextlib import ExitStack

import concourse.bass as bass
import concourse.mybir as mybir
import concourse.tile as tile
from concourse import bacc

P = 128          # partitions / inter channels
C = 256          # input channels
F32 = mybir.dt.float32
F32R = mybir.dt.float32r
F16 = mybir.dt.float16
BF16 = mybir.dt.bfloat16
I16 = mybir.dt.int16
AF = mybir.ActivationFunctionType
ALU = mybir.AluOpType
CSHIFT = 40.0    # global score shift before exp (softmax-invariant)
A_SCALE = 184.66496414625282          # 2^7 * log2(e); theta pre-scale
B_SHIFT = 16256.0 - 5.5 - CSHIFT * A_SCALE  # Schraudolph offset (C=5.5)

B_FULL = 8
H_FULL = 64
W_FULL = 64
N_FULL = H_FULL * W_FULL

WARM_MMS = 40    # junk matmuls to trip the PE HAM to 8/8 before real work
DVE_EXP = {10, 22}   # chunks whose exp runs on DVE (q>=1)
# GpSimd col-sum offload measured counterproductive: its tensor_tensor adds
# run at ~2.4us AND slow concurrent DVE adds 700->1110ns (SBUF port
# contention), so all column sums stay on DVE.
GP_ACC = frozenset()


def build_nc(N=N_FULL, NQ=1024):
    """Build the single-core Bass module (SPMD: same NEFF on all 8 cores)."""
    MC = N // P                   # 32 key chunks
    NQn = N // NQ                 # 4 query quarters
    NB = NQ // 512                # 2 512-col blocks per quarter
    NCB = N // 1024               # 4 1024-col proj blocks

    nc = bacc.Bacc("TRN2", target_bir_lowering=False, debug=False)

    xh_d = nc.dram_tensor("xh", [C, N], F16, kind="ExternalInput").ap()
    xpb_d = nc.dram_tensor("xpb", [C, N], F32R, kind="ExternalInput").ap()
    wtT_d = nc.dram_tensor("wtT", [P, 2 * P], F16, kind="ExternalInput").ap()
    wpT_d = nc.dram_tensor("wpT", [P, 2 * P], F16, kind="ExternalInput").ap()
    wgT_d = nc.dram_tensor("wgT", [P, 2 * P], F16, kind="ExternalInput").ap()
    wWT_d = nc.dram_tensor("wWT", [P, C], F16, kind="ExternalInput").ap()
    bt_d = nc.dram_tensor("bt", [P, 1], F32, kind="ExternalInput").ap()
    out_d = nc.dram_tensor("out", [C, N], F32R, kind="ExternalOutput").ap()

    xh_v = xh_d.rearrange("(k p) n -> k p n", p=P)
    xpb_v = xpb_d.rearrange("(k p) n -> k p n", p=P)
    out_v = out_d.rearrange("(k p) n -> k p n", p=P)

    with tile.TileContext(nc) as tc, ExitStack() as ctx:
        const = ctx.enter_context(tc.tile_pool(name="const", bufs=1))
        big = ctx.enter_context(tc.tile_pool(name="big", bufs=1))
        work = ctx.enter_context(tc.tile_pool(name="work", bufs=1))
        ps = ctx.enter_context(tc.tile_pool(name="ps", bufs=1, space="PSUM"))

        # ---- constants ----
        wtT_sb = const.tile([P, 2, P], F16, name="wtT_sb")
        wpT_sb = const.tile([P, 2, P], F16, name="wpT_sb")
        wgT_sb = const.tile([P, 2, P], F16, name="wgT_sb")
        wWT_sb = const.tile([P, C], F16, name="wWT_sb")
        bt_sb = const.tile([P, 1], F32, name="bt_sb")
        ones_sb = const.tile([P, P], BF16, name="ones_sb")
        cshift_sb = const.tile([P, 1], F32, name="cshift_sb")
        nc.vector.memset(cshift_sb[:], -CSHIFT)
        nc.vector.memset(ones_sb[:], 1.0)

        # weights on the scalar queue (small; lands before first use)
        nc.scalar.dma_start(wtT_sb[:], wtT_d.rearrange("p (k i) -> p k i", k=2))
        nc.scalar.dma_start(wpT_sb[:], wpT_d.rearrange("p (k i) -> p k i", k=2))
        nc.scalar.dma_start(bt_sb[:], bt_d)
        nc.scalar.dma_start(wgT_sb[:], wgT_d.rearrange("p (k i) -> p k i", k=2))
        nc.scalar.dma_start(wWT_sb[:], wWT_d)

        # x halves on two parallel queues: k=0 via sync, k=1 via gpsimd
        xh_sb = big.tile([P, 2, N], F16, name="xh_sb")
        for b in range(8):
            sl = slice(b * 512, (b + 1) * 512)
            nc.sync.dma_start(xh_sb[:, 0, sl], xh_v[0, :, sl])
        for b in range(8):
            sl = slice(b * 512, (b + 1) * 512)
            nc.gpsimd.dma_start(xh_sb[:, 1, sl], xh_v[1, :, sl])

        # residual (+bW', folded on host) pre-store: one DRAM->DRAM copy,
        # queued behind the x stream on the sync queue. The LAST quarter is
        # stored plainly instead, with its residual slice added from SBUF.
        NPRE = N - NQ
        nc.sync.dma_start(out_d[:, 0:NPRE], xpb_d[:, 0:NPRE])
        xpbq_sb = big.tile([P, 2, NQ], F32R, name="xpbq_sb")
        for k in range(2):
            nc.sync.dma_start(xpbq_sb[:, k], xpb_v[k, :, NPRE:N])

        # per-block tiles (single writer each -> exact matmul weight deps)
        th_t = [big.tile([P, 1024], F16, name=f"th{i}") for i in range(NCB)]
        ph_t = [big.tile([P, 1024], F16, name=f"ph{i}") for i in range(NCB)]
        g_t = [big.tile([P, 4, P], BF16, name=f"g{i}") for i in range(8)]

        # ---- PE warmup: trip HAM to 8/8 while the first x blocks land ----
        warm_ps = ps.tile([P, P], F32, tag="s", bufs=2, name="warm_ps")
        for _ in range(WARM_MMS):
            nc.tensor.matmul(warm_ps[:], ones_sb[:], ones_sb[:],
                             start=True, stop=True, skip_group_check=True)

        def proj_block(dst_t, w_sb, cb, tag, engine):
            """theta/phi projection for 1024-col block cb; extract on
            `engine` ('act'|'dve'); theta also gets (+bt)*A fused."""
            is_th = dst_t is th_t
            p_ps = ps.tile([P, 1024], F32, tag=tag, bufs=2 if tag == "s"
                           else None, name="p_ps")
            for h in range(2):
                hsl = slice(cb * 1024 + h * 512, cb * 1024 + (h + 1) * 512)
                for k in range(2):
                    nc.tensor.matmul(p_ps[:, h * 512:(h + 1) * 512],
                                     w_sb[:, k], xh_sb[:, k, hsl],
                                     start=(k == 0), stop=(k == 1))
            if is_th:
                nc.vector.tensor_scalar(dst_t[cb][:], p_ps[:],
                                        bt_sb[:, 0:1], A_SCALE,
                                        op0=ALU.add, op1=ALU.mult)
            elif engine == "act":
                nc.scalar.activation(dst_t[cb][:], p_ps[:], AF.Identity)
            else:
                nc.vector.tensor_copy(dst_t[cb][:], p_ps[:])

        def g_block(t):
            """g for key chunks 4t..4t+3 (x block t)."""
            g_ps = ps.tile([P, 4, P], F32, tag="wy", name="g_ps")
            for j in range(4):
                msl = slice((4 * t + j) * P, (4 * t + j + 1) * P)
                for k in range(2):
                    nc.tensor.matmul(g_ps[:, j], xh_sb[:, k, msl],
                                     wgT_sb[:, k], start=(k == 0),
                                     stop=(k == 1))
            nc.vector.tensor_copy(g_t[t][:], g_ps[:])

        # critical first blocks: th0 (extract on DVE) + ph0 (extract on ACT)
        # run in the "s" slots before the attention chain starts.
        proj_block(th_t, wtT_sb, 0, "s", "dve")
        proj_block(ph_t, wpT_sb, 0, "s", "act")

        # deferred prep, interleaved into q0's chunk loop (the tile
        # scheduler bakes a per-engine order from its cost-model sim, so
        # slack-tolerant work must be EMITTED late or its stalls leak into
        # the baked order ahead of the S matmuls)
        prep = {0: lambda: g_block(0),
                1: lambda: g_block(1),
                3: lambda: proj_block(ph_t, wpT_sb, 1, "wy", "dve"),
                5: lambda: g_block(2),
                7: lambda: g_block(3),
                9: lambda: proj_block(ph_t, wpT_sb, 2, "wy", "dve"),
                11: lambda: g_block(4),
                13: lambda: g_block(5),
                15: lambda: proj_block(ph_t, wpT_sb, 3, "wy", "dve"),
                17: lambda: g_block(6),
                19: lambda: g_block(7),
                21: lambda: proj_block(th_t, wtT_sb, 1, "wy", "dve"),
                23: lambda: proj_block(th_t, wtT_sb, 2, "wy", "dve"),
                25: lambda: proj_block(th_t, wtT_sb, 3, "wy", "dve")}

        YLAG = 2  # y-matmul emission lag (chunks), protects PE queue order

        def epi_sums(E):
            """Deferred epilogue of quarter E: sums matmul (merging both
            accumulators), reciprocal, and the fp16 normalized yt."""
            sum_ps = ps.tile([P, NQ], F32, tag="wy", name="sum_ps")
            for b in range(NB):
                bsl = slice(b * 512, (b + 1) * 512)
                accs = [E["acc_d"]] + ([E["acc_g"]] if GP_ACC else [])
                for i, a in enumerate(accs):
                    nc.tensor.matmul(sum_ps[:, bsl], ones_sb[:], a[:, bsl],
                                     start=(i == 0), stop=(i == len(accs) - 1),
                                     skip_group_check=True)
            recip_sb = work.tile([P, NQ], F32, tag="recip", bufs=2,
                                 name="recip_sb")
            nc.vector.reciprocal_approx_fast(recip_sb[:], sum_ps[:])
            nc.vector.tensor_mul(E["ytn"][:], E["yt"][:], recip_sb[:])

        def epi_wy(E, h):
            """Deferred epilogue: wW projection half h + accumulate-store."""
            q = E["q"]
            wy_ps = ps.tile([P, NQ], F32, tag="wy", name="wy_ps")
            for b in range(NB):
                bsl = slice(b * 512, (b + 1) * 512)
                nc.tensor.matmul(wy_ps[:, bsl], wWT_sb[:, h * P:(h + 1) * P],
                                 E["ytn"][:, bsl], start=True, stop=True)
            o_sb = work.tile([P, NQ], F32R, tag="ow", bufs=2, name="o_sb")
            nc.vector.tensor_copy(o_sb[:], wy_ps[:])
            nc.gpsimd.dma_start(out_v[h, :, q * NQ:(q + 1) * NQ], o_sb[:],
                                accum_op=ALU.add)

        pending = None  # previous quarter's epilogue state

        for q in range(NQn):
            last = (q == NQn - 1)
            y_ps = ps.tile([P, NQ], F32, tag="y", name="y_ps")
            acc_d = work.tile([P, NQ], BF16, tag="acc_d", bufs=2, name="acc_d")
            acc_g = (work.tile([P, NQ], BF16, tag="acc_g", bufs=2,
                               name="acc_g") if GP_ACC else None)
            exps = {}
            gp_first = min(GP_ACC) if GP_ACC else None

            def y_mms(mc):
                e = exps.pop(mc)
                for b in range(NB):
                    bsl = slice(b * 512, (b + 1) * 512)
                    nc.tensor.matmul(y_ps[:, bsl], g_t[mc // 4][:, mc % 4],
                                     e[:, bsl],
                                     start=(mc == 0), stop=(mc == MC - 1),
                                     skip_group_check=True)

            def chunk(mc, on_dve):
                s_ps = ps.tile([P, NQ], F32, tag="wy" if on_dve else "s",
                               bufs=None if on_dve else 2, name="s_ps")
                msl = slice((mc % 8) * P, (mc % 8 + 1) * P)
                for b in range(NB):
                    bsl = slice(b * 512, (b + 1) * 512)
                    nc.tensor.matmul(
                        s_ps[:, bsl], ph_t[mc // 8][:, msl],
                        th_t[q][:, b * 512:(b + 1) * 512],
                        start=True, stop=True)
                exp_sb = work.tile([P, NQ], BF16, tag="exp", bufs=12,
                                   name="exp_sb")
                if on_dve:
                    # Schraudolph: bf16 bits of e^(S-40) via one DVE op
                    nc.vector.tensor_scalar(exp_sb[:].bitcast(I16), s_ps[:],
                                            B_SHIFT, 0.0,
                                            op0=ALU.add, op1=ALU.max)
                else:
                    nc.scalar.activation(exp_sb[:], s_ps[:], AF.Exp,
                                         bias=cshift_sb[:, 0:1],
                                         scale=1.0 / A_SCALE)
                exps[mc] = exp_sb
                if mc == 0:
                    nc.vector.tensor_copy(acc_d[:], exp_sb[:])
                else:
                    nc.vector.tensor_add(acc_d[:], acc_d[:], exp_sb[:])

            # DVE-exp chunks are EMITTED two chunks late (full slack): the
            # baked PE order then never holds their S matmuls - which wait
            # on the DVE-paced "wy" rotation - ahead of ACT-chunk S matmuls.
            for mc in range(MC):
                if not (q > 0 and mc in DVE_EXP):
                    chunk(mc, False)
                if mc - 2 in DVE_EXP and q > 0:
                    chunk(mc - 2, True)
                ynow = mc - YLAG
                if ynow >= 0 and not (q > 0 and ynow in DVE_EXP):
                    y_mms(ynow)
                ydve = mc - 4   # DVE chunks' y matmuls get extra slack
                if q > 0 and ydve in DVE_EXP:
                    y_mms(ydve)
                if q == 0 and mc in prep:
                    prep[mc]()
                if pending is not None:
                    if mc == 2:
                        epi_sums(pending)
                    elif mc == 5:
                        epi_wy(pending, 0)
                    elif mc == 8:
                        epi_wy(pending, 1)
                        pending = None
            for mc in sorted(exps):
                y_mms(mc)

            # y extraction frees the y banks for the next quarter; the rest
            # of the epilogue is deferred into the next quarter's loop.
            yt_sb = work.tile([P, NQ], F32R, tag="yt", bufs=2, name="yt_sb")
            ytn_sb = work.tile([P, NQ], F16, tag="ytn", bufs=2, name="ytn_sb")
            if last:
                nc.scalar.activation(yt_sb[:], y_ps[:], AF.Identity)
            else:
                nc.vector.tensor_copy(yt_sb[:], y_ps[:])
            pending = {"q": q, "acc_d": acc_d, "acc_g": acc_g,
                       "yt": yt_sb, "ytn": ytn_sb}

        # ---- tail: the last quarter's epilogue, plain stores, pipelined
        # in 512-col pieces (the DVE chain recip->ytn->o-add is critical)
        E = pending
        q = E["q"]
        sum_ps = ps.tile([P, NQ], F32, tag="wy", name="sum_ps")
        for b in range(NB):
            bsl = slice(b * 512, (b + 1) * 512)
            nc.tensor.matmul(sum_ps[:, bsl], ones_sb[:], E["acc_d"][:, bsl],
                             start=True, stop=True, skip_group_check=True)
        recip_sb = work.tile([P, NQ], F32, tag="recip", bufs=2,
                             name="recip_sb")
        for b in range(NB):
            bsl = slice(b * 512, (b + 1) * 512)
            nc.vector.reciprocal_approx_fast(recip_sb[:, bsl],
                                             sum_ps[:, bsl])
            nc.vector.tensor_mul(E["ytn"][:, bsl], E["yt"][:, bsl],
                                 recip_sb[:, bsl])
        for h in range(2):
            wy_ps = ps.tile([P, NQ], F32, tag="wy", name="wy_ps")
            for b in range(NB):
                bsl = slice(b * 512, (b + 1) * 512)
                nc.tensor.matmul(wy_ps[:, bsl], wWT_sb[:, h * P:(h + 1) * P],
                                 E["ytn"][:, bsl], start=True, stop=True)
            for b in range(NB):
                bsl = slice(b * 512, (b + 1) * 512)
                o_sb = work.tile([P, 512], F32R, tag="o", bufs=4, name="o_sb")
                nc.vector.tensor_add(o_sb[:], wy_ps[:, bsl],
                                     xpbq_sb[:, h, bsl])
                nc.sync.dma_start(
                    out_v[h, :, q * NQ + b * 512: q * NQ + (b + 1) * 512],
                    o_sb[:])

        # junk matmuls keep the PE HAM at 8/8 through the tail epilogue
        tailwarm_ps = ps.tile([P, P], F32, tag="s", bufs=2, name="tailwarm")
        for _ in range(36):
            nc.tensor.matmul(tailwarm_ps[:], ones_sb[:], ones_sb[:],
                             start=True, stop=True, skip_group_check=True)

    nc.compile()
    return nc


_CACHE = {}


def _built(key=(N_FULL, 1024)):
    if key not in _CACHE:
        _CACHE[key] = build_nc(*key)
    return _CACHE[key]


def make_in_maps(x, wg, bg, wt, bt, wp, bp, wW, bW):
    """Host-side prep: per-core input dicts (core b <- batch b)."""
    x = np.asarray(x, np.float32)
    B, C_, H, W = x.shape
    N = H * W
    xf = np.ascontiguousarray(x.reshape(B, C_, N))
    wg, bg, wt, bt, wp, bp, wW, bW = [
        np.asarray(a, np.float32) for a in (wg, bg, wt, bt, wp, bp, wW, bW)]

    def pack(w):  # (128, C) conv weight -> partition-major lhsT, fp16
        return np.ascontiguousarray(
            w.T.reshape(2, P, P).transpose(1, 0, 2).reshape(P, 2 * P)
        ).astype(np.float16)

    bWp = (wW @ bg + bW).astype(np.float32)       # fold bg into bW
    shared = {
        "wtT": pack(wt), "wpT": pack(wp), "wgT": pack(wg),
        "wWT": np.ascontiguousarray(wW.T).astype(np.float16),
        "bt": bt.reshape(P, 1).copy(),
    }
    return [{"xh": np.ascontiguousarray(xf[b]).astype(np.float16),
             "xpb": np.ascontiguousarray(xf[b] + bWp[:, None]),
             **shared} for b in range(B)]


def kernel(x, wg, bg, wt, bt, wp, bp, wW, bW):
    from concourse.bass_utils import run_bass_kernel_spmd

    B, C_, H, W = np.asarray(x).shape
    in_maps = make_in_maps(x, wg, bg, wt, bt, wp, bp, wW, bW)
    nc = _built()
    res = run_bass_kernel_spmd(nc, in_maps, core_ids=list(range(B)))
    out = np.stack([res.results[b]["out"] for b in range(B)])
    return out.reshape(B, C_, H, W).astype(np.float32)
